# revision 76
# baseline (speedup 1.0000x reference)
"""Trainium2 Bass kernel for Transformer-XL style relative-position multi-head
self-attention (nn_MultiHeadedSelfAttention_35588099015524).

Sharding: batch (B=8) is data-parallel across the 8 NeuronCores; no collectives.

Math (same as baseline v2): the relative shift is eliminated exactly via
    bd^T[j,i] = sum_c F[j,c] * A'[i,c]
with A' a per-position rotation of G = Wpos_h^T q_v, split into an exact HIGH
band (f 0..63, 128 ch) and a numerically-exact rank-64 SVD of the LOW band.
Scores fuse AC + bd_low in one 128-deep matmul (KU = [k ; U^T]) plus one
128-deep HIGH matmul; computed transposed with a ones-column in V so softmax
sums ride along.

v3 scheduling rewrite (this file):
  - input DMAs only on the SP + DVE queues (ACT queue stays clean so the
    Q-bias/exp chain starts immediately); weights batched; WPT packed to
    [D, D] (head pairs share a 128-row block).
  - v_bias - u_bias folded into a host constant added during the G psum
    evacuation (ACT Identity+bias) -- the qvT tiles and their DVE adds are
    gone; G's moving operand is qub's q-rows.
  - PSUM: shared 4-slot [128,512] ring (scores, G, B, projections) + 4-slot
    [65,512] AV ring, so neither scores nor AV ever wait on an evacuation.
  - softmax-sum reciprocal via DVE reciprocal_approx_fast (no Ln/Exp chain).
  - per-pair endgame: h1's AV runs jt0-5 / h0-tail / znorm(h0) / h1 jt6-7 /
    znorm(h1); every znorm chain drains under the next pair's first jts.
  - output projection: all 8 chunks' ncnk 0-2 partials hide under pair 3's
    endgame; only the 8 zT3 matmuls remain after the last znorm; out in bf16.
"""

import sys

sys.path.insert(0, "/opt/trn_rl_repo")

from contextlib import ExitStack  # noqa: E402

import numpy as np  # noqa: E402
import ml_dtypes  # noqa: E402

import concourse.bass as bass  # noqa: E402
from concourse import bacc, library_config  # noqa: E402
import concourse.tile as tile  # noqa: E402
from concourse import mybir  # noqa: E402
from concourse.bass_utils import run_bass_kernel_spmd  # noqa: E402

# Force every ACT function we use (Exp/Copy/Identity) to resolve to the single
# "natural_log_exp_and_others" table set -- otherwise the table-load pass
# flip-flops between sets (~2.7us per ACT_TABLE_LOAD).
import concourse.hw_specs as _hs  # noqa: E402
import concourse.bacc as _bacc_mod  # noqa: E402

if not getattr(_hs, "_act_tables_pinned", False):
    _orig_gat = _hs.get_activation_tables

    def _pinned_gat(arch):
        tabs = _orig_gat(arch)
        keep = "natural_log_exp_and_others"
        pin = {mybir.ActivationFunctionType.Exp,
               mybir.ActivationFunctionType.Ln,
               mybir.ActivationFunctionType.Copy,
               mybir.ActivationFunctionType.Identity}
        if keep in tabs and pin <= tabs[keep]:
            for k in tabs:
                if k != keep:
                    tabs[k] = tabs[k] - pin
        return tabs

    _hs.get_activation_tables = _pinned_gat
    _bacc_mod.get_activation_tables = _pinned_gat
    _hs._act_tables_pinned = True

B, T, D = 8, 1024, 512
H, DH = 8, 64
NCORES = 8
SCALE = 1.0 / np.sqrt(DH)
F0 = 64          # split: f < F0 exact, f >= F0 via rank-R SVD
R = 64           # SVD rank (exact to ~1e-15 at R=64)

F32 = mybir.dt.float32
BF16 = mybir.dt.bfloat16

MM_MODE = "bf16"


def _np_dt(mode):
    return ml_dtypes.bfloat16 if mode == "bf16" else np.float32


def _mm_dt(mode):
    return BF16 if mode == "bf16" else mybir.dt.float32r


def build_nc(mode=MM_MODE):
    """Build the per-core Bass module (identical program on all 8 cores)."""
    DT = _mm_dt(mode)
    nc = bacc.Bacc("TRN2", target_bir_lowering=False, debug=False)

    # ---- DRAM parameters (per core) ----
    xsT_d = nc.declare_dram_parameter("xsT", [D, T], DT, isOutput=False)
    wq_d = nc.declare_dram_parameter("Wq", [D, D], DT, isOutput=False)
    wk_d = nc.declare_dram_parameter("Wk", [D, D], DT, isOutput=False)
    wv_d = nc.declare_dram_parameter("Wv", [D, D], DT, isOutput=False)
    wpt_d = nc.declare_dram_parameter("WPT", [D, D], DT, isOutput=False)
    wout_d = nc.declare_dram_parameter("Wout", [D, D], DT, isOutput=False)
    ubt_d = nc.declare_dram_parameter("ubT", [128, 4], F32, isOutput=False)
    gb_d = nc.declare_dram_parameter("gbT", [128, 32], F32, isOutput=False)
    ct_d = nc.declare_dram_parameter("CT", [256, T], DT, isOutput=False)
    st_d = nc.declare_dram_parameter("ST", [256, T], DT, isOutput=False)
    fh_d = nc.declare_dram_parameter("FH", [128, T], DT, isOutput=False)
    ut_d = nc.declare_dram_parameter("UT2", [128, T], DT, isOutput=False)
    mt_d = nc.declare_dram_parameter("MT", [128, 3 * R], DT, isOutput=False)
    out_d = nc.declare_dram_parameter("out", [T, D], DT, isOutput=True)

    Exp = mybir.ActivationFunctionType.Exp
    Copy = mybir.ActivationFunctionType.Copy
    Ident = mybir.ActivationFunctionType.Identity
    MUL = mybir.AluOpType.mult
    ADD = mybir.AluOpType.add
    SUB = mybir.AluOpType.subtract

    with tile.TileContext(nc) as tc, ExitStack() as ctx:
        cpool = ctx.enter_context(tc.tile_pool(name="consts", bufs=1))
        gpool = ctx.enter_context(tc.tile_pool(name="gwork", bufs=2))
        mpool = ctx.enter_context(tc.tile_pool(name="mids", bufs=1))
        apool = ctx.enter_context(tc.tile_pool(name="attn", bufs=2))
        opool = ctx.enter_context(tc.tile_pool(name="osb", bufs=4))
        rpool = ctx.enter_context(tc.tile_pool(name="recip", bufs=2))
        rbpool = ctx.enter_context(tc.tile_pool(name="recipb", bufs=2))
        ps_a = ctx.enter_context(tc.tile_pool(name="ps_a", bufs=5,
                                              space="PSUM"))
        ps_z = ctx.enter_context(tc.tile_pool(name="ps_z", bufs=3,
                                              space="PSUM"))

        # ---- input DMAs: ALL on the SP queue (only SP/ACT have HWDGE
        # rings; ACT must stay clean so the bias/G-copy/exp chain never
        # queues behind a 1.3us DMA issue).  Batched per tensor, issued in
        # criticality order.
        warm = cpool.tile([128, 512], DT, tag="warm", name="warm")
        nc.vector.memset(warm[:], 0.0)

        xsT_tile = gpool.tile([128, 4 * T], DT, tag="xsT", name="xsT")
        wq_tile = cpool.tile([128, 4 * D], DT, tag="wq", name="wq")
        ubt = cpool.tile([128, 4], F32, tag="ubt")
        gbt = cpool.tile([128, 32], F32, tag="gbt")

        def load_wide(dram, rows, cols, tag):
            nblk = rows // 128
            t = cpool.tile([128, nblk * cols], DT, tag=tag, name=tag)
            nc.sync.dma_start(
                t[:].rearrange("p (c i) -> p c i", c=nblk),
                dram[:, :].rearrange("(c p) i -> p c i", p=128))
            return [t[:, c * cols:(c + 1) * cols] for c in range(nblk)]

        nc.sync.dma_start(
            xsT_tile[:].rearrange("p (c i) -> p c i", c=4),
            xsT_d[:, :].rearrange("(c p) i -> p c i", p=128))
        xsT = [xsT_tile[:, c * T:(c + 1) * T] for c in range(4)]
        nc.sync.dma_start(
            wq_tile[:].rearrange("p (c i) -> p c i", c=4),
            wq_d[:, :].rearrange("(c p) i -> p c i", p=128))
        wq = [wq_tile[:, c * D:(c + 1) * D] for c in range(4)]
        nc.sync.dma_start(ubt[:], ubt_d[:, :])
        nc.sync.dma_start(gbt[:], gb_d[:, :])
        wpt = load_wide(wpt_d, D, D, "wpt")
        wk = load_wide(wk_d, D, D, "wk")
        ct = load_wide(ct_d, 256, T, "ct")
        st = load_wide(st_d, 256, T, "st")
        wv = load_wide(wv_d, D, D, "wv")
        mt = cpool.tile([128, 3 * R], DT, tag="mt", name="mt")
        nc.sync.dma_start(mt[:], mt_d[:, :])
        fh = cpool.tile([128, T], DT, tag="fh", name="fh")
        nc.sync.dma_start(fh[:], fh_d[:, :])
        mtm = mt[:, 0:R]          # [128 c=(sin f64-127|cos f64-127), R]
        mt1 = mt[:, R:2 * R]      # [128 c=sin f128-255, R]
        mt3 = mt[:, 2 * R:3 * R]  # [128 c=cos f128-255, R]

        # KU_h = [k_h (64 rows) ; U^T (64 rows)] (order flips with h parity
        # so the K-proj psum copy stays same-partition); loaded just-in-time
        # in head order
        KU = [cpool.tile([128, T], DT, tag=f"KU{h}", name=f"KU{h}")
              for h in range(H)]
        for h in range(H):
            row = (1 - h % 2) * 64
            nc.sync.dma_start(KU[h][row:row + 64, :], ut_d[row:row + 64, :])
        wout = load_wide(wout_d, D, D, "wout")

        qub = [cpool.tile([128, T], DT, tag=f"qub{h}", name=f"qub{h}")
               for h in range(H)]
        zT = [cpool.tile([128, T], DT, tag=f"zT{c}", name=f"zT{c}")
              for c in range(4)]
        vp = cpool.tile([128, 8 * 520], DT, tag="vp")

        nc.gpsimd.load_library(library_config.attn)
        # ones columns for the softmax-sum trick (V overwrites on top)
        nc.gpsimd.memset(vp[:], 1.0)

        # PE warm-up covering the batched xsT/wq DMA window (keeps the
        # p-state ramp hot so the Q projection runs at full clock)
        wp0 = ps_a.tile([128, 512], F32, tag="a", name="warmp")
        for w in range(14):
            nc.tensor.matmul(wp0[:], warm[:, 0:128], warm[:, 0:512],
                             start=True, stop=True)

        # ---- Q projection (+u bias -> qub) ----
        def emit_q_chunk(n):
            pq = [ps_a.tile([128, 512], F32, tag="a", name=f"qp{n}_{ic}")
                  for ic in range(2)]
            for kc in range(4):
                for ic in range(2):
                    nc.tensor.matmul(
                        pq[ic][:],
                        wq[kc][:, n * 128:(n + 1) * 128],
                        xsT[kc][:, ic * 512:(ic + 1) * 512],
                        start=(kc == 0),
                        stop=(kc == 3),
                    )
            for ic in range(2):
                o = ic * 512
                nc.scalar.activation(qub[2 * n][0:64, o:o + 512],
                                     pq[ic][0:64, :],
                                     Ident, bias=ubt[0:64, n:n + 1])
                nc.vector.tensor_scalar_add(qub[2 * n + 1][64:128, o:o + 512],
                                            pq[ic][64:128, :],
                                            ubt[64:128, n:n + 1])

        # ---- per-head G -> rope -> highT/midT/B pipeline ----
        def emit_g_piece(h, pc, g, evac):
            """G matmul for piece pc (= cc*2 + icnk) of head h.  Moving
            operand is qub's q-rows (u-biased); the u->v bias fix is a host
            constant added during the psum evacuation (ACT or DVE)."""
            cc, icnk = pc // 2, pc % 2
            row = (h % 2) * 64
            p = ps_a.tile([128, 512], F32, tag="a", name=f"gp{h}_{pc}")
            nc.tensor.matmul(
                p[:],
                wpt[h // 2][row:row + 64, cc * 128:(cc + 1) * 128],
                qub[h][row:row + 64, icnk * 512:(icnk + 1) * 512],
                start=True,
                stop=True,
            )
            dst = g[:, cc * 1024 + icnk * 512: cc * 1024 + icnk * 512 + 512]
            bias = gbt[:, h * 4 + cc:h * 4 + cc + 1]
            if evac == "act":
                nc.scalar.activation(dst, p[:], Ident, bias=bias)
            else:
                nc.vector.tensor_scalar_add(dst, p[:], bias)

        def emit_rope(h, g):
            """A' chunks in order cc0, cc2, cc1, cc3 (= As f0-127, Ac f0-127
            first, so the hi/mid swap DMAs fire mid-rope); all DVE."""
            ap = gpool.tile([128, 4096], DT, tag="aprime", name=f"a2_{h}")
            tmp = gpool.tile([128, T], DT, tag="tmp")
            tmp2 = gpool.tile([128, T], DT, tag="tmp")
            for cc in (0, 2, 1, 3):
                freq = cc % 2
                sin_blk = cc < 2
                ga = g[:, cc * 1024:(cc + 1) * 1024]
                pcn = (cc + 2) % 4
                gb = g[:, pcn * 1024:(pcn + 1) * 1024]
                dst = ap[:, cc * 1024:(cc + 1) * 1024]
                nc.vector.tensor_tensor(tmp[:], ga, ct[freq][:], op=MUL)
                nc.vector.tensor_tensor(tmp2[:], gb, st[freq][:], op=MUL)
                nc.vector.tensor_tensor(dst, tmp[:], tmp2[:],
                                        op=(ADD if sin_blk else SUB))
            return ap

        def emit_swaps(h, ap):
            """highT = [As f0-63; Ac f0-63], midT = [As f64-127; Ac f64-127]
            via partition-remapping SBUF->SBUF DMAs."""
            hi = gpool.tile([128, T], DT, tag=f"highT{h % 2}", name=f"hi{h}")
            mid = mpool.tile([128, T], DT, tag=f"midT{h % 2}", name=f"mid{h}")
            nc.sync.dma_start(hi[0:64, :], ap[0:64, 0:1024])
            nc.sync.dma_start(hi[64:128, :], ap[0:64, 2048:3072])
            nc.sync.dma_start(mid[0:64, :], ap[64:128, 0:1024])
            nc.sync.dma_start(mid[64:128, :], ap[64:128, 2048:3072])
            return hi, mid

        def emit_b(h, ap, mid):
            """B_h = M @ A'_low; even heads accumulate on psum rows 64-127,
            odd on 0-63, so the qub copy stays same-partition (Pool)."""
            r0 = 64 if h % 2 == 0 else 0
            pb = [ps_a.tile([128, 512], F32, tag="a", name=f"bp{h}_{icnk}")
                  for icnk in range(2)]
            # icnk-interleaved so each stationary (mtm/mt1/mt3) loads once
            for pi, stat in enumerate((mtm, mt1, mt3)):
                for icnk in range(2):
                    o = icnk * 512
                    mv = (mid[:, o:o + 512],
                          ap[:, 1024 + o:1536 + o],
                          ap[:, 3072 + o:3584 + o])[pi]
                    nc.tensor.matmul(pb[icnk][r0:r0 + 64, :], stat[:], mv,
                                     start=(pi == 0), stop=(pi == 2))
            for icnk in range(2):
                o = icnk * 512
                if h % 2 == 0:
                    nc.scalar.activation(qub[h][r0:r0 + 64, o:o + 512],
                                         pb[icnk][r0:r0 + 64, :], Copy)
                else:
                    nc.vector.tensor_copy(qub[h][r0:r0 + 64, o:o + 512],
                                          pb[icnk][r0:r0 + 64, :])

        def emit_v_chunk(jt):
            """V projection for s-rows 128jt..128jt+127, strided into vp
            (Pool evacuation)."""
            p = ps_a.tile([128, 512], F32, tag="a", name=f"vp{jt}")
            for kc in range(4):
                nc.tensor.matmul(
                    p[:],
                    xsT[kc][:, jt * 128:(jt + 1) * 128],
                    wv[kc][:],
                    start=(kc == 0),
                    stop=(kc == 3),
                )
            dst = vp[:, jt * 520:(jt + 1) * 520].rearrange(
                "p (h x) -> p h x", h=8)[:, :, 0:64]
            src = p[:, :].rearrange("p (h x) -> p h x", h=8)
            if jt % 2 == 0:
                nc.scalar.activation(dst, src, Copy)
            else:
                nc.vector.tensor_copy(dst, src)

        def emit_k_chunk(n, odd_dve=True):
            pk = [ps_a.tile([128, 512], F32, tag="a", name=f"kp{n}_{ic}")
                  for ic in range(2)]
            for kc in range(4):
                for ic in range(2):
                    nc.tensor.matmul(
                        pk[ic][:],
                        wk[kc][:, n * 128:(n + 1) * 128],
                        xsT[kc][:, ic * 512:(ic + 1) * 512],
                        start=(kc == 0),
                        stop=(kc == 3),
                    )
            for ic in range(2):
                o = ic * 512
                nc.scalar.activation(KU[2 * n][0:64, o:o + 512],
                                     pk[ic][0:64, :], Copy)
                if odd_dve:
                    nc.vector.tensor_copy(KU[2 * n + 1][64:128, o:o + 512],
                                          pk[ic][64:128, :])
                else:
                    nc.scalar.activation(KU[2 * n + 1][64:128, o:o + 512],
                                         pk[ic][64:128, :], Copy)

        def emit_av_mm(h, icnk, jt, zp, attnT):
            nc.tensor.matmul(
                zp[0:65, :],
                vp[:, jt * 520 + 65 * h: jt * 520 + 65 * h + 65],
                attnT[:, jt * 1024 + icnk * 512:
                      jt * 1024 + icnk * 512 + 512],
                start=(jt == 0),
                stop=(jt == 7),
            )

        Ln = mybir.ActivationFunctionType.Ln

        def emit_znorm(h, icnk, zp):
            """zT[h] <- zp rows 0-63 scaled by 1/sums (row 64): ACT Ln ->
            ACT Exp(-x) -> Pool broadcast (row 0 only!) -> DVE multiply."""
            lns = rpool.tile([1, 512], F32, tag="lnsf")
            nc.scalar.activation(lns[:], zp[64:65, :], Ln)
            rec = rpool.tile([1, 512], F32, tag="lr")
            nc.scalar.activation(rec[:], lns[:], Exp, scale=-1.0)
            recb = rbpool.tile([64, 512], F32, tag="recb")
            nc.gpsimd.partition_broadcast(recb[:], rec[0:1, :])
            row = (h % 2) * 64
            dst = zT[h // 2][row:row + 64, icnk * 512:(icnk + 1) * 512]
            nc.vector.tensor_tensor(dst, zp[0:64, :], recb[:], op=MUL)

        # ---- prologue ----
        # Pair 0's two heads only need nchunk0's bias, so its G pieces run
        # right after Q chunk 0; both heads' G evacs ride ACT here (rope is
        # waiting on them).  K/V interleave to keep the PE fed while the
        # DVE rope chain (the prologue critical path) drains.
        g00 = gpool.tile([128, 4096], DT, tag="g", name="g0")
        g01 = gpool.tile([128, 4096], DT, tag="g", name="g1")
        # piece order (0,1,4,5,...) so rope's first half (cc0+cc2) can start
        # after only 4 evacuations
        emit_q_chunk(0)
        for pc in (0, 1, 4, 5, 2, 3, 6, 7):
            emit_g_piece(0, pc, g00, "act")
        for pc in (0, 1, 4, 5, 2, 3, 6, 7):
            emit_g_piece(1, pc, g01, "act")
        emit_q_chunk(1)
        emit_q_chunk(2)
        emit_q_chunk(3)
        emit_k_chunk(0)
        a00 = emit_rope(0, g00)
        hi00, mid00 = emit_swaps(0, a00)
        emit_k_chunk(1)
        emit_b(0, a00, mid00)
        a01 = emit_rope(1, g01)
        hi01, mid01 = emit_swaps(1, a01)
        emit_k_chunk(2)
        emit_k_chunk(3)
        emit_v_chunk(0)
        emit_v_chunk(1)
        emit_b(1, a01, mid01)
        hi_cur = (hi00, hi01)

        # output-projection partial staging (ncnk 0-2, chunks 0-3 only),
        # filled during pair 3's endgame
        opart = [cpool.tile([128, 512], DT, tag=f"opart{it}",
                            name=f"opart{it}") for it in range(4)]

        # ---- pair loop ----
        for m in range(4):
            h0, h1 = 2 * m, 2 * m + 1
            attnT0 = apool.tile([128, 8192], DT, tag="attnT", name=f"at{h0}")
            attnT1 = apool.tile([128, 8192], DT, tag="attnT", name=f"at{h1}")
            zp0 = ps_z.tile([65, 512], F32, tag="z", name=f"zp0_{h0}")
            zp1 = ps_z.tile([65, 512], F32, tag="z", name=f"zp1_{h0}")
            if m + 1 < 4:
                g_n0 = gpool.tile([128, 4096], DT, tag="g", name=f"g{h0 + 2}")
                g_n1 = gpool.tile([128, 4096], DT, tag="g", name=f"g{h1 + 2}")

            for jt in range(8):
                jc = jt * 128
                # at jt0 the next pair's first G pieces go FIRST: the ring
                # slots are all free (jt7's scores are exp'd) and rope's
                # inputs arrive earlier
                if m + 1 < 4 and jt == 0:
                    for pc in (0, 1, 4, 5):
                        emit_g_piece(h0 + 2, pc, g_n0, "act")
                # 8 score matmuls: fused AC+bd_low then HIGH, h0 then h1
                # (3 ldweights/jt: KU[h0], KU[h1], fh)
                ps_c = []
                for h in (h0, h1):
                    hidx = h - h0
                    for ic in range(2):
                        p = ps_a.tile([128, 512], F32, tag="a",
                                      name=f"s{h}_{jt}_{ic}")
                        ps_c.append((h, hidx, ic, p))
                for h, hidx, ic, p in ps_c:
                    nc.tensor.matmul(
                        p[:], KU[h][:, jc:jc + 128],
                        qub[h][:, ic * 512:(ic + 1) * 512],
                        start=True, stop=False)
                for h, hidx, ic, p in ps_c:
                    nc.tensor.matmul(
                        p[:], fh[:, jc:jc + 128],
                        hi_cur[hidx][:, ic * 512:(ic + 1) * 512],
                        start=False, stop=True)

                # AV: h0 of this pair lags one jt (h1 runs in the endgame)
                if jt > 0:
                    emit_av_mm(h0, 0, jt - 1, zp0, attnT0)
                    emit_av_mm(h0, 1, jt - 1, zp1, attnT0)
                if m == 0 and jt + 2 < 8:
                    emit_v_chunk(jt + 2)

                # spread next pair's G/rope/swaps/B across this pair's loop:
                # h0' pieces (ACT evac) jt0-1, h1' pieces (DVE evac) jt1-2,
                # rope h0' from jt2, B h0' + rope h1' at jt4, B h1' at jt6
                if m + 1 < 4:
                    if jt == 1:
                        for pc in (2, 3, 6, 7):
                            emit_g_piece(h0 + 2, pc, g_n0, "act")
                        for pc in (0, 1, 4, 5):
                            emit_g_piece(h1 + 2, pc, g_n1, "dve")
                    elif jt == 2:
                        for pc in (2, 3, 6, 7):
                            emit_g_piece(h1 + 2, pc, g_n1, "dve")
                        a_n0 = emit_rope(h0 + 2, g_n0)
                        hi_n0, mid_n0 = emit_swaps(h0 + 2, a_n0)
                    elif jt == 4:
                        emit_b(h0 + 2, a_n0, mid_n0)
                        a_n1 = emit_rope(h1 + 2, g_n1)
                        hi_n1, mid_n1 = emit_swaps(h1 + 2, a_n1)
                    elif jt == 6:
                        emit_b(h1 + 2, a_n1, mid_n1)

                # exp into attnT (ACT), one op per [128,512] psum chunk
                for h, hidx, ic, p in ps_c:
                    at = attnT0 if hidx == 0 else attnT1
                    nc.scalar.activation(
                        at[:, jt * 1024 + ic * 512: jt * 1024 + ic * 512 + 512],
                        p[:], Exp, scale=float(SCALE))

            # ---- pair endgame ----
            # h0's trailing AVs, then znorm(zp0) runs under h1's icnk0 AV
            # block; zp1c (the 4th z tile on a 3-ring) is first written only
            # after znorm(zp0)'s multiply frees its slot.
            zp0c = ps_z.tile([65, 512], F32, tag="z", name=f"zp0_{h1}")
            zp1c = ps_z.tile([65, 512], F32, tag="z", name=f"zp1_{h1}")
            emit_av_mm(h0, 0, 7, zp0, attnT0)
            emit_av_mm(h0, 1, 7, zp1, attnT0)
            emit_znorm(h0, 0, zp0)
            for jt in range(8):
                emit_av_mm(h1, 0, jt, zp0c, attnT1)
            emit_znorm(h0, 1, zp1)
            for jt in range(8):
                emit_av_mm(h1, 1, jt, zp1c, attnT1)
            emit_znorm(h1, 0, zp0c)
            emit_znorm(h1, 1, zp1c)

            if m + 1 < 4:
                hi_cur = (hi_n0, hi_n1)

        # ---- output projection ----
        # Chunks 0-3: ncnk 0-2 partials staged to opart while the last
        # znorm chain drains, ncnk3 + DVE add at the end.  Chunks 4-7: all
        # four ncnk accumulate in one held psum (the znorm is done by then)
        # and evacuate with a plain ACT copy -- no adds.
        for it in range(4):
            p = ps_a.tile([128, 512], F32, tag="a", name=f"opp{it}")
            for ncnk in range(3):
                nc.tensor.matmul(
                    p[:],
                    zT[ncnk][:, it * 128:(it + 1) * 128],
                    wout[ncnk][:],
                    start=(ncnk == 0),
                    stop=(ncnk == 2),
                )
            nc.scalar.activation(opart[it][:], p[:], Copy)
        phold = []
        for it in range(4, 8):
            p = ps_a.tile([128, 512], F32, tag="a", name=f"opp{it}")
            for ncnk in range(3):
                nc.tensor.matmul(
                    p[:],
                    zT[ncnk][:, it * 128:(it + 1) * 128],
                    wout[ncnk][:],
                    start=(ncnk == 0),
                    stop=False,
                )
            phold.append(p)
        for i, it in enumerate(range(4, 8)):
            p = phold[i]
            nc.tensor.matmul(
                p[:], zT[3][:, it * 128:(it + 1) * 128], wout[3][:],
                start=False, stop=True)
            osb = opool.tile([128, 512], DT, tag="osb")
            nc.scalar.activation(osb[:], p[:], Copy)
            (nc.sync, nc.scalar)[it % 2].dma_start(
                out_d[it * 128:(it + 1) * 128, :], osb[:])
        for it in range(4):
            p = ps_a.tile([128, 512], F32, tag="a", name=f"op{it}")
            nc.tensor.matmul(
                p[:],
                zT[3][:, it * 128:(it + 1) * 128],
                wout[3][:],
                start=True,
                stop=True,
            )
            osb = opool.tile([128, 512], DT, tag="osb")
            nc.vector.tensor_tensor(osb[:], p[:], opart[it][:], op=ADD)
            (nc.sync, nc.scalar)[it % 2].dma_start(
                out_d[it * 128:(it + 1) * 128, :], osb[:])

    nc.compile()
    _dedup_ldweights(nc)
    return nc


def _ldw_range(inst):
    """(base_partition, n_partitions) of an InstLdweights' stationary AP."""
    try:
        ba = inst.ins[0].bass_ap
        return (int(ba.base_partition()), int(ba.partition_size()))
    except Exception:
        return None


def _dedup_ldweights(nc):
    """Drop an InstLdweights when the weights for its row-strip range are
    already loaded (same stationary AP, no intervening overlapping load)."""
    removed = 0
    for fn in nc.m.functions:
        for blk in fn.blocks:
            last = {}  # (base, n) -> sig
            newlist = []
            for inst in blk.instructions:
                if isinstance(inst, mybir.InstLdweights):
                    sig = str(inst.ins[0])
                    rng = _ldw_range(inst)
                    si = inst.sync_info
                    clean = si is None or (
                        len(si.on_wait) == 0 and len(si.on_update) == 0)
                    if clean and rng is not None and last.get(rng) == sig:
                        removed += 1
                        continue
                    if rng is None:
                        last.clear()
                    else:
                        b0, n0 = rng
                        for (b, n) in list(last):
                            if not (b + n <= b0 or b0 + n0 <= b):
                                del last[(b, n)]
                        last[rng] = sig
                    newlist.append(inst)
                else:
                    newlist.append(inst)
            blk.instructions[:] = newlist
    return removed


_SVD_CACHE = {}


def _svd_tables():
    """Host constants for the split-BD decomposition (input-independent)."""
    if "v" in _SVD_CACHE:
        return _SVD_CACHE["v"]
    kk = np.arange(256, dtype=np.float64)
    w = np.exp(-np.log(10000.0) * (2.0 * kk) / D)
    j = np.arange(T, dtype=np.float64)
    sin_t = np.sin(np.outer(w, j))
    cos_t = np.cos(np.outer(w, j))
    # high table FH = [sin f0-63 ; cos f0-63]
    FH = np.concatenate([sin_t[:F0], cos_t[:F0]], axis=0)
    # low block SVD
    Flow = np.concatenate([sin_t[F0:].T, cos_t[F0:].T], axis=1)  # (T, 384)
    U, S, Vt = np.linalg.svd(Flow, full_matrices=False)
    U, S, Vt = U[:, :R], S[:R], Vt[:R]
    M = S[:, None] * Vt                                         # (R, 384)
    # stationaries, rows = channel order of the corresponding moving tile
    MTM = np.concatenate([M[:, 0:64], M[:, 192:256]], axis=1).T   # (128, R)
    MT1 = M[:, 64:192].T                                          # (128, R)
    MT3 = M[:, 256:384].T                                         # (128, R)
    MT = np.concatenate([MTM, MT1, MT3], axis=1)                  # (128, 3R)
    UT2 = np.concatenate([U.T, U.T], axis=0)                      # (128, T)
    _SVD_CACHE["v"] = (sin_t.astype(np.float32), cos_t.astype(np.float32),
                       FH.astype(np.float32), MT.astype(np.float32),
                       UT2.astype(np.float32))
    return _SVD_CACHE["v"]


def make_host_inputs(xs, Wq, Wk, Wv, Wpos, Wout, u_bias, v_bias, mode=MM_MODE):
    """Build the per-core input maps (host-side layout prep only)."""
    npdt = _np_dt(mode)
    sin_t, cos_t, FH, MT, UT2 = _svd_tables()

    perm = np.concatenate([np.arange(0, D, 2), np.arange(1, D, 2)])
    WPTn = Wpos[perm, :].T                                     # (hd, c)
    # packed WPT: block m rows 0-63 = head 2m, rows 64-127 = head 2m+1
    WPT = np.zeros((D, D), np.float32)
    for h in range(H):
        WPT[(h // 2) * 128 + (h % 2) * 64:
            (h // 2) * 128 + (h % 2) * 64 + 64, :] = \
            WPTn[h * DH:(h + 1) * DH, :]

    ubT = np.ascontiguousarray(
        u_bias.reshape(-1).astype(np.float32).reshape(4, 128).T)
    # G bias correction: G uses qu (=q+u) as moving, true G needs q+v;
    # gb[:, h*4+cc] = ((v-u)_h @ WPos_h-permuted)[cc*128:(cc+1)*128]
    gb = np.zeros((128, 32), np.float32)
    for h in range(H):
        c_h = (v_bias[h] - u_bias[h]).astype(np.float64) @ \
            WPTn[h * DH:(h + 1) * DH, :].astype(np.float64)     # (512,)
        for cc in range(4):
            gb[:, h * 4 + cc] = c_h[cc * 128:(cc + 1) * 128]

    shared = {
        "Wq": np.ascontiguousarray(Wq).astype(npdt),
        "Wk": np.ascontiguousarray(Wk).astype(npdt),
        "Wv": np.ascontiguousarray(Wv).astype(npdt),
        "WPT": WPT.astype(npdt),
        "Wout": np.ascontiguousarray(Wout).astype(npdt),
        "ubT": ubT,
        "gbT": gb,
        "CT": np.ascontiguousarray(cos_t).astype(npdt),
        "ST": np.ascontiguousarray(sin_t).astype(npdt),
        "FH": np.ascontiguousarray(FH).astype(npdt),
        "UT2": np.ascontiguousarray(UT2).astype(npdt),
        "MT": np.ascontiguousarray(MT).astype(npdt),
    }
    in_maps = []
    for b in range(B):
        m = dict(shared)
        m["xsT"] = np.ascontiguousarray(xs[b].T).astype(npdt)
        in_maps.append(m)
    return in_maps


_NC_CACHE = {}


def get_nc(mode=MM_MODE):
    if mode not in _NC_CACHE:
        _NC_CACHE[mode] = build_nc(mode)
    return _NC_CACHE[mode]


def _numpy_reference(xs, mask, Wq, Wk, Wv, Wpos, Wout, u_bias, v_bias):
    """Exact (fp32 numpy) fallback for non-all-ones masks."""
    b, t, _ = xs.shape
    pos = np.arange(-(t - 1), t, dtype=np.float32)[:, None]
    inv_freq = np.exp(-np.log(10000.0) *
                      np.arange(0, D, 2, dtype=np.float32) / D)
    angv = pos * inv_freq[None, :]
    pe = np.stack([np.sin(angv), np.cos(angv)], axis=-1).reshape(pos.shape[0], D)
    q = (xs @ Wq).reshape(b, t, H, DH).transpose(0, 2, 1, 3)
    k = (xs @ Wk).reshape(b, t, H, DH).transpose(0, 2, 1, 3)
    v = (xs @ Wv).reshape(b, t, H, DH).transpose(0, 2, 1, 3)
    p = (pe @ Wpos).reshape(-1, H, DH).transpose(1, 0, 2)
    q_u = q + u_bias[None, :, None, :]
    q_v = q + v_bias[None, :, None, :]
    ac = np.einsum("bhtd,bhsd->bhts", q_u, k)
    bd = np.einsum("bhtd,hld->bhtl", q_v, p)
    bdp = np.pad(bd, ((0, 0), (0, 0), (0, 0), (1, 0)))
    l = bd.shape[-1]
    bd = bdp.reshape(b, H, l + 1, t)[:, :, 1:, :].reshape(b, H, t, l)[..., :t]
    scores = (ac + bd) * SCALE
    m = (mask[:, None, :, :] == 0)
    scores = np.where(m, -np.inf, scores)
    scores = scores - scores.max(axis=-1, keepdims=True)
    e = np.exp(scores)
    attn = e / e.sum(axis=-1, keepdims=True)
    attn = np.where(m, 0.0, attn)
    z = np.einsum("bhts,bhsd->bthd", attn, v).reshape(b, t, H * DH)
    return (z @ Wout).astype(np.float32)


def kernel(xs, mask, Wq, Wk, Wv, Wpos, Wout, u_bias, v_bias):
    xs = np.asarray(xs, dtype=np.float32)
    mask = np.asarray(mask)
    Wq = np.asarray(Wq, dtype=np.float32)
    Wk = np.asarray(Wk, dtype=np.float32)
    Wv = np.asarray(Wv, dtype=np.float32)
    Wpos = np.asarray(Wpos, dtype=np.float32)
    Wout = np.asarray(Wout, dtype=np.float32)
    u_bias = np.asarray(u_bias, dtype=np.float32)
    v_bias = np.asarray(v_bias, dtype=np.float32)

    if not np.all(mask != 0):
        # the on-device kernel assumes the (spec-pinned) all-ones mask
        return _numpy_reference(xs, mask, Wq, Wk, Wv, Wpos, Wout,
                                u_bias, v_bias)

    nc = get_nc(MM_MODE)
    in_maps = make_host_inputs(xs, Wq, Wk, Wv, Wpos, Wout, u_bias, v_bias,
                               MM_MODE)
    res = run_bass_kernel_spmd(nc, in_maps, core_ids=list(range(NCORES)))
    out = np.stack([np.asarray(res.results[b]["out"], dtype=np.float32)
                    for b in range(B)], axis=0)
    return out


if __name__ == "__main__":
    nc = build_nc()
    print("build ok")
